# revision 1
# baseline (speedup 1.0000x reference)
"""Trainium2 Bass kernel for EnhancedMultiHeadAttention.

Data-parallel over batch: B=256 split as 32 batches per core across 8 cores.
Per core, 8 chunks of 256 tokens flow through a software pipeline.

Key optimizations over the fp16 baseline:
  - The global-branch projections (g_q, g_k, g_v) and the global half of the
    output projection run in fp8e4 (e4m3) with DoubleRow perf mode: two
    128-deep contraction tiles per pass, 2x PE throughput. Weights are
    pre-scaled x32/x64 host-side so e4m3 sees unit-scale data; descales are
    folded into the exp() scale and the v-projection drain.
  - All weights resident in SBUF, loaded once; every DMA-able tensor is
    pre-shuffled host-side into its exact SBUF layout so DMAs move long
    contiguous per-partition runs (strided descriptors are slow).
  - Chunk c's local-score matmuls+drains are interleaved instruction-by-
    instruction with chunk c-1's softmax/AV so no engine queue head-of-line
    blocks the exp chain; the rel-bias T-store + skew-add DMAs fire early
    and overlap the rest of the iteration.
  - attn^T comes from PE transposes arranged so every AV matmul runs with
    operands at PE-array row 0: odd-parity v batches are mirrored down to
    partitions 0:64 by one SBUF-SBUF DMA per chunk. (Back-to-back 64-row
    matmuls that alternate row position 0<->64 hang the hardware.)
  - Relative-position bias: T = q @ REL_EXT.T computed into the score PSUM,
    drained together with qk, stored once per chunk, and skew-added via a
    DRAM roundtrip with a diagonal (per-l shifted) read.

Output convention: PSUM of the final projection is x512 (wo pre-scaled);
the host wrapper divides by 512 after gathering.
"""

import sys

sys.path.insert(0, "/opt/trn_rl_repo")

import math
from contextlib import ExitStack

import numpy as np
import ml_dtypes

import concourse.bass as bass
import concourse.tile as tile
from concourse import bacc, mybir
from concourse.bass_utils import run_bass_kernel_spmd
from concourse.masks import make_identity

F16 = mybir.dt.float16
F32 = mybir.dt.float32
F8 = mybir.dt.float8e4
DR = mybir.MatmulPerfMode.DoubleRow
EXP = mybir.ActivationFunctionType.Exp
CPY = mybir.ActivationFunctionType.Copy

B, L, D = 256, 64, 1024
H, HD = 16, 64
HG, HDG = 8, 128
MAXREL = 32
NCORES = 8
BC = B // NCORES  # batches per core
NTOK = BC * L  # tokens per core
CH = 256  # tokens per chunk
NB = CH // L  # batches per chunk (4)
NCHUNK = NTOK // CH  # 8
KT = D // 128  # contraction tiles (8)
OT = D // 128  # output feature tiles (8)
NQ = CH * H // (2 * L)  # head-batch pairs per chunk (32)
NG = 2 * NB + NB  # softmax groups per chunk (8 local + 4 global)

SCALE_G = 1.0 / (1024.0 * math.sqrt(HDG))  # global exp descale
OUT_SCALE = 512.0  # final psum scale, host divides


def _emit(nc, tc, ctx, io):
    pconst = ctx.enter_context(tc.tile_pool(name="pconst", bufs=1))
    px = ctx.enter_context(tc.tile_pool(name="px", bufs=2))
    pact = ctx.enter_context(tc.tile_pool(name="pact", bufs=1))
    psmall = ctx.enter_context(tc.tile_pool(name="psmall", bufs=2))
    patt = ctx.enter_context(tc.tile_pool(name="patt", bufs=3))
    pout = ctx.enter_context(tc.tile_pool(name="pout", bufs=2))
    pp_proj = ctx.enter_context(tc.tile_pool(name="pp_proj", bufs=2, space="PSUM"))
    pp_big = ctx.enter_context(tc.tile_pool(name="pp_big", bufs=2, space="PSUM"))
    pp_sc = ctx.enter_context(tc.tile_pool(name="pp_sc", bufs=2, space="PSUM"))
    pp_av = ctx.enter_context(tc.tile_pool(name="pp_av", bufs=2, space="PSUM"))
    pdram = ctx.enter_context(tc.tile_pool(name="pdram", bufs=2, space="DRAM"))

    st = [dict() for _ in range(NCHUNK)]

    def load_x(c):
        x16 = px.tile([128, 3, KT, CH], F16, tag="x16", name="x16")
        nc.sync.dma_start(x16[:], io["x16"][c])
        x8 = px.tile([128, 3, KT, CH], F8, tag="x8", name="x8")
        nc.scalar.dma_start(x8[:], io["x8"][c])
        st[c]["x16"] = x16
        st[c]["x8"] = x8

    # ---- persistent weights, spread across queues so chunk 0 starts early
    load_x(0)
    w_q = pconst.tile([128, KT, D], F16, tag="w_q")
    nc.sync.dma_start(w_q[:], io["w_q"][:])
    w_k = pconst.tile([128, KT, D], F16, tag="w_k")
    nc.sync.dma_start(w_k[:], io["w_k"][:])
    w8x = pconst.tile([128, 4, KT, D], F8, tag="w8x")  # gq, gk, gv, w2
    nc.scalar.dma_start(w8x[:], io["w8x"][:])
    w16r = pconst.tile([128, 2, KT, D], F16, tag="w16r")  # v, o
    nc.gpsimd.dma_start(w16r[:], io["w16r"][:])
    relt = pconst.tile([128, 127], F16, tag="relt")
    nc.sync.dma_start(relt[0:64, :], io["relt"][:])
    nc.sync.dma_start(relt[64:128, :], io["relt"][:])
    ident = pconst.tile([128, 128], F16, tag="ident")
    make_identity(nc, ident[:])
    wt = {
        "w_q": w_q,
        "w_k": w_k,
        "w_v": w16r[:, 0],
        "w_o": w16r[:, 1],
        "w_gq": w8x[:, 0],
        "w_gk": w8x[:, 1],
        "w_gv": w8x[:, 2],
        "w_2": w8x[:, 3],
    }

    # ---- projections ----
    def proj_t16(xt, w, dst, drain):
        # dst[p, oi, t] f16: transposed output (features on partitions)
        for op in range(OT // 2):
            ps = pp_proj.tile([128, 2, CH], F32, tag="pj")
            for ii in range(2):
                oi = 2 * op + ii
                for ki in range(KT):
                    nc.tensor.matmul(
                        ps[:, ii, :],
                        w[:, ki, oi * 128 : (oi + 1) * 128],
                        xt[:, ki, :],
                        start=(ki == 0),
                        stop=(ki == KT - 1),
                    )
            drain(dst[:, 2 * op : 2 * op + 2, :], ps[:])

    def proj_t8(xt8, w8, dst, drain):
        for op in range(OT // 2):
            ps = pp_proj.tile([128, 2, CH], F32, tag="pj")
            for ii in range(2):
                oi = 2 * op + ii
                for kp in range(KT // 2):
                    nc.tensor.matmul(
                        ps[:, ii, :],
                        w8[:, 2 * kp : 2 * kp + 2, oi * 128 : (oi + 1) * 128],
                        xt8[:, 2 * kp : 2 * kp + 2, :],
                        start=(kp == 0),
                        stop=(kp == KT // 2 - 1),
                        perf_mode=DR,
                    )
            drain(dst[:, 2 * op : 2 * op + 2, :], ps[:])

    def proj_n16(xt, w, dst):
        # dst[p, ts, o] f16: plain layout (tokens on partitions)
        for ts in range(NB // 2):
            for oh in range(2):
                ps = pp_big.tile([128, 512], F32, tag="mm")
                for ki in range(KT):
                    nc.tensor.matmul(
                        ps[:],
                        xt[:, ki, ts * 128 : (ts + 1) * 128],
                        w[:, ki, oh * 512 : (oh + 1) * 512],
                        start=(ki == 0),
                        stop=(ki == KT - 1),
                    )
                nc.scalar.copy(out=dst[:, ts, oh * 512 : (oh + 1) * 512], in_=ps[:])

    def proj_n8(xt8, w8, dst, scale):
        for ts in range(NB // 2):
            for oh in range(2):
                ps = pp_big.tile([128, 512], F32, tag="mm")
                for kp in range(KT // 2):
                    nc.tensor.matmul(
                        ps[:],
                        xt8[:, 2 * kp : 2 * kp + 2, ts * 128 : (ts + 1) * 128],
                        w8[:, 2 * kp : 2 * kp + 2, oh * 512 : (oh + 1) * 512],
                        start=(kp == 0),
                        stop=(kp == KT // 2 - 1),
                        perf_mode=DR,
                    )
                nc.scalar.activation(
                    dst[:, ts, oh * 512 : (oh + 1) * 512], ps[:], CPY, scale=scale
                )

    def drain_dve(dst, src):
        nc.vector.tensor_copy(out=dst, in_=src)

    def drain_act(dst, src):
        nc.scalar.copy(out=dst, in_=src)

    def mirror(src, tag):
        # odd-parity batches' v rows moved down to partitions 0:64 so every
        # AV matmul can run at PE-array row 0
        dup = pact.tile([64, NB // 2, D], F16, tag=tag, name=tag)
        nc.sync.dma_start(dup[:], src[64:128, :, :])
        return dup

    def proj_qk(c):
        qt = pact.tile([128, OT, CH], F16, tag="qt", name="qt")
        proj_t16(st[c]["x16"][:, 0], wt["w_q"], qt, drain_dve)
        kt_ = pact.tile([128, OT, CH], F16, tag="kt", name="kt")
        proj_t16(st[c]["x16"][:, 1], wt["w_k"], kt_, drain_dve)
        st[c].update(qt=qt, kt=kt_)

    def proj_rest(c):
        x16, x8 = st[c]["x16"], st[c]["x8"]
        qgt = pact.tile([128, OT, CH], F16, tag="qgt", name="qgt")
        proj_t8(x8[:, 0], wt["w_gq"], qgt, drain_dve)
        kgt = pact.tile([128, OT, CH], F16, tag="kgt", name="kgt")
        proj_t8(x8[:, 1], wt["w_gk"], kgt, drain_dve)
        vt = pact.tile([128, NB // 2, D], F16, tag="vt", name="vt")
        proj_n16(x16[:, 2], wt["w_v"], vt)
        vgt = pact.tile([128, NB // 2, D], F16, tag="vgt", name="vgt")
        proj_n8(x8[:, 2], wt["w_gv"], vgt, 0.25)
        st[c].update(qgt=qgt, kgt=kgt, vt=vt, vgt=vgt,
                     vdup=mirror(vt, "vdup"), vgdup=mirror(vgt, "vgdup"))

    # ---- local scores, one pt-pair at a time ----
    def scores_alloc(c):
        qksb = pact.tile([128, NQ, 191], F16, tag="qksb", name="qksb")
        tdr = pdram.tile([2, 64, NQ, 127], F16, tag="tdr")
        st[c]["qksb"] = qksb
        st[c]["tdr"] = tdr

    def scores_pt(c, pt):
        qt, kt_, qksb = st[c]["qt"], st[c]["kt"], st[c]["qksb"]
        b = pt // 8
        bcols = slice(b * L, (b + 1) * L)
        # full-bank psum tile (2048B/partition) so matmuls stay in-bank
        sc = pp_sc.tile([128, 2, 256], F32, tag="sc")
        for u in range(2):
            j = (pt + u) % 8
            qa = qt[0:64, j, bcols]
            qb = qt[64:128, j, bcols]
            nc.tensor.matmul(sc[0:64, u, 0:64], qa, kt_[0:64, j, bcols],
                             start=True, stop=True)
            nc.tensor.matmul(sc[64:128, u, 0:64], qb, kt_[64:128, j, bcols],
                             start=True, stop=True)
            nc.tensor.matmul(sc[0:64, u, 64:191], qa, relt[0:64, :],
                             start=True, stop=True)
            nc.tensor.matmul(sc[64:128, u, 64:191], qb, relt[64:128, :],
                             start=True, stop=True)
        nc.scalar.copy(out=qksb[:, pt : pt + 2, :], in_=sc[:, :, 0:191])

    def scores_fin(c):
        qksb, tdr = st[c]["qksb"], st[c]["tdr"]
        tap = tdr[:]
        # one T store for the whole chunk:
        # src qksb[(pair,l), q, 64:191] -> tdr[pair, l, q, j] ((q,j) contiguous)
        dst = bass.AP(
            tap.tensor,
            tap.offset,
            [[64 * NQ * 127, 2], [NQ * 127, 64], [1, NQ * 127]],
        )
        nc.scalar.dma_start(dst, qksb[:, :, 64:191])
        # skew-read: qksb[pair*64+l, q, r] += tdr[pair, l, q, r-l+63]
        for pair in range(2):
            src = bass.AP(
                tap.tensor,
                tap.offset + 63 + pair * 64 * NQ * 127,
                [[NQ * 127 - 1, 64], [127, NQ], [1, 64]],
            )
            nc.gpsimd.dma_start(
                qksb[pair * 64 : (pair + 1) * 64, :, 0:64],
                src,
                accum_op=mybir.AluOpType.add,
            )

    def scores_global(c):
        qgt, kgt = st[c]["qgt"], st[c]["kgt"]
        gsb = pact.tile([128, 4 * NB, 64], F16, tag="gsb", name="gsb")
        for g in range(NB):
            bcols = slice(g * L, (g + 1) * L)
            sc = pp_av.tile([128, 512], F32, tag="av")
            for i in range(4):
                for pair in range(2):
                    hg = 2 * i + pair
                    nc.tensor.matmul(
                        sc[pair * 64 : (pair + 1) * 64, i * 64 : (i + 1) * 64],
                        qgt[:, hg, bcols],
                        kgt[:, hg, bcols],
                        start=True,
                        stop=True,
                    )
            nc.scalar.copy(
                out=gsb[:, 4 * g : 4 * g + 4, :],
                in_=sc[:, 0:256].rearrange("p (i c) -> p i c", i=4),
            )
        st[c]["gsb"] = gsb

    # ---- softmax + AV, one group (4 q-pairs) at a time ----
    def softmax_block(src_ap, scale, av):
        # av: this group's [128, 512] f32 psum bank; cols 256:512 (viewed as
        # f16 cols 512:1024) hold the 4 transposed-attn tiles [64, 128].
        esb = psmall.tile([128, 4, 64], F16, tag="esb")
        if scale is None:
            nc.scalar.activation(esb[:], src_ap, EXP)
        else:
            nc.scalar.activation(esb[:], src_ap, EXP, scale=scale)
        sums = psmall.tile([128, 4], F32, tag="sums")
        nc.vector.tensor_reduce(sums[:], esb[:], mybir.AxisListType.X,
                                mybir.AluOpType.add)
        rcp = psmall.tile([128, 4], F32, tag="rcp")
        nc.vector.reciprocal(rcp[:], sums[:])
        attn = psmall.tile([128, 4, 64], F16, tag="attn")
        nc.vector.tensor_tensor(
            attn[:], esb[:], rcp[:, :, None].to_broadcast([128, 4, 64]),
            mybir.AluOpType.mult,
        )
        trv = av[:].bitcast(F16)  # [128, 1024] f16 view of the bank
        for i in range(4):
            nc.tensor.transpose(
                trv[0:64, 512 + i * 128 : 512 + (i + 1) * 128],
                attn[:, i, :],
                ident[:],
            )
        # att4[r, i, l2]: i-th q-pair's attn^T, [64 r, 128 (lo-l | hi-l)]
        att4 = patt.tile([64, 4, 128], F16, tag="att4", name="att4")
        nc.scalar.copy(
            out=att4[:],
            in_=trv[0:64, 512:1024].rearrange("p (i c) -> p i c", i=4),
        )
        return att4

    def smx_local_g(c, g):
        qksb, vt, vdup, lt = st[c]["qksb"], st[c]["vt"], st[c]["vdup"], st[c]["lt"]
        b = g // 2
        j0 = 4 * (g % 2)
        bcols = slice(b * L, (b + 1) * L)
        av = pp_av.tile([128, 512], F32, tag="av")
        att4 = softmax_block(qksb[:, 4 * g : 4 * g + 4, 0:64], None, av)
        vsrc = vt if (b % 2) == 0 else vdup
        for i in range(4):
            for pair in range(2):
                h = 2 * (j0 + i) + pair
                nc.tensor.matmul(
                    av[pair * 64 : (pair + 1) * 64, i * 64 : (i + 1) * 64],
                    vsrc[0:64, b // 2, h * 64 : (h + 1) * 64],
                    att4[:, i, pair * 64 : (pair + 1) * 64],
                    start=True,
                    stop=True,
                )
        nc.vector.tensor_copy(
            out=lt[:, j0 : j0 + 4, bcols],
            in_=av[:, 0:256].rearrange("p (i c) -> p i c", i=4),
        )

    def smx_global_g(c, g):
        gsb, vgt, vgdup, gt = st[c]["gsb"], st[c]["vgt"], st[c]["vgdup"], st[c]["gt"]
        b = g
        bcols = slice(b * L, (b + 1) * L)
        av0 = pp_av.tile([128, 512], F32, tag="av")
        att4 = softmax_block(gsb[:, 4 * g : 4 * g + 4, :], SCALE_G, av0)
        vsrc = vgt if (b % 2) == 0 else vgdup
        for half in range(2):
            av = av0 if half == 0 else pp_av.tile([128, 512], F32, tag="av")
            for k in range(4):
                hg = 4 * half + k
                i, pair = hg // 2, hg % 2
                for hh in range(2):
                    nc.tensor.matmul(
                        av[hh * 64 : (hh + 1) * 64, k * 64 : (k + 1) * 64],
                        vsrc[0:64, b // 2,
                             hg * 128 + hh * 64 : hg * 128 + (hh + 1) * 64],
                        att4[:, i, pair * 64 : (pair + 1) * 64],
                        start=True,
                        stop=True,
                    )
            nc.vector.tensor_copy(
                out=gt[:, 4 * half : 4 * half + 4, bcols],
                in_=av[:, 0:256].rearrange("p (k c) -> p k c", k=4),
            )

    def smx_alloc(c):
        st[c]["lt"] = pact.tile([128, KT, CH], F16, tag="lt", name="lt")
        st[c]["gt"] = pact.tile([128, KT, CH], F8, tag="gt", name="gt")

    def smx_g(c, m):
        if m < 2 * NB:
            smx_local_g(c, m)
        else:
            smx_global_g(c, m - 2 * NB)

    def final_chunk(c):
        tok0 = c * CH
        lt, gt = st[c]["lt"], st[c]["gt"]
        for ts in range(NB // 2):
            osb = pout.tile([128, 1024], F32, tag="out")
            for oh in range(2):
                ps = pp_big.tile([128, 512], F32, tag="mm")
                for ki in range(KT):
                    nc.tensor.matmul(
                        ps[:],
                        lt[:, ki, ts * 128 : (ts + 1) * 128],
                        wt["w_o"][:, ki, oh * 512 : (oh + 1) * 512],
                        start=(ki == 0),
                        stop=False,
                    )
                for kp in range(KT // 2):
                    nc.tensor.matmul(
                        ps[:],
                        gt[:, 2 * kp : 2 * kp + 2, ts * 128 : (ts + 1) * 128],
                        wt["w_2"][:, 2 * kp : 2 * kp + 2, oh * 512 : (oh + 1) * 512],
                        start=False,
                        stop=(kp == KT // 2 - 1),
                        perf_mode=DR,
                    )
                nc.vector.tensor_copy(out=osb[:, oh * 512 : (oh + 1) * 512], in_=ps[:])
            nc.scalar.dma_start(
                io["out"][tok0 + ts * 128 : tok0 + (ts + 1) * 128, :], osb[:]
            )
        st[c].clear()

    # ---- pipeline ----
    LEAD = NQ // 2 - NG  # score pt-pairs emitted before interleaving (4)
    proj_qk(0)
    scores_alloc(0)
    for pt in range(0, NQ, 2):
        scores_pt(0, pt)
    scores_fin(0)
    proj_rest(0)
    scores_global(0)
    for c in range(1, NCHUNK):
        load_x(c)
        proj_qk(c)
        scores_alloc(c)
        smx_alloc(c - 1)
        for pt in range(0, NQ, 2):
            scores_pt(c, pt)
        scores_fin(c)
        for m in range(NG):
            smx_g(c - 1, m)
        proj_rest(c)
        final_chunk(c - 1)
        scores_global(c)
    smx_alloc(NCHUNK - 1)
    for m in range(NG):
        smx_g(NCHUNK - 1, m)
    final_chunk(NCHUNK - 1)


_NC_CACHE = {}


def _get_module():
    if "nc" not in _NC_CACHE:
        nc = bacc.Bacc("TRN2", target_bir_lowering=False, debug=False)
        io = {}
        io["x16"] = nc.dram_tensor(
            "x16", [NCHUNK, 128, 3, KT, CH], F16, kind="ExternalInput"
        ).ap()
        io["x8"] = nc.dram_tensor(
            "x8", [NCHUNK, 128, 3, KT, CH], F8, kind="ExternalInput"
        ).ap()
        io["w_q"] = nc.dram_tensor("w_q", [128, KT, D], F16, kind="ExternalInput").ap()
        io["w_k"] = nc.dram_tensor("w_k", [128, KT, D], F16, kind="ExternalInput").ap()
        io["w16r"] = nc.dram_tensor(
            "w16r", [128, 2, KT, D], F16, kind="ExternalInput"
        ).ap()
        io["w8x"] = nc.dram_tensor(
            "w8x", [128, 4, KT, D], F8, kind="ExternalInput"
        ).ap()
        io["relt"] = nc.dram_tensor("relt", [64, 127], F16, kind="ExternalInput").ap()
        io["out"] = nc.dram_tensor("out", [NTOK, D], F32, kind="ExternalOutput").ap()
        with tile.TileContext(nc) as tc, ExitStack() as ctx:
            _emit(nc, tc, ctx, io)
        nc.compile()
        _NC_CACHE["nc"] = nc
    return _NC_CACHE["nc"]


def _shuf_w(w):
    # [D(=ki*128+p), O] -> [p, ki, O]
    return np.ascontiguousarray(w.reshape(KT, 128, -1).transpose(1, 0, 2))


def _shuf_x(x3):
    # [3, D(=ki*128+p), NTOK(=c*CH+t)] -> [c, p, 3, ki, t]
    return np.ascontiguousarray(
        x3.reshape(3, KT, 128, NCHUNK, CH).transpose(3, 2, 0, 1, 4)
    )


def _prepare_in_maps(inputs):
    f32 = lambda name: np.asarray(inputs[name], np.float32)
    f8 = lambda a: a.astype(ml_dtypes.float8_e4m3fn)

    S = 32.0
    wq_t = f32("Wq").T.astype(np.float16)
    wk_t = (f32("Wk").T / math.sqrt(HD)).astype(np.float16)
    wv_t = f32("Wv").T.astype(np.float16)
    g_in = f32("g_in_w")
    wgq8 = f8(g_in[0:D].T * S)
    wgk8 = f8(g_in[D : 2 * D].T * S)  # no sqrt fold; folded into SCALE_G
    wgv8 = f8(g_in[2 * D : 3 * D].T * S)
    wo = f32("Wo")
    gow = f32("g_out_w")
    wo_t = (0.7 * OUT_SCALE * wo.T).astype(np.float16)
    w2_8 = f8(0.3 * 64.0 * (gow.T @ wo.T))  # gt is x8 => 8*64 = OUT_SCALE

    for bname in ("bq", "bk", "bv", "bo", "g_in_b", "g_out_b"):
        assert not np.any(f32(bname)), f"nonzero bias {bname} not supported"

    rel_k = f32("rel_k")
    ext_ids = np.clip(np.arange(127) - 63, -MAXREL, MAXREL) + MAXREL
    relt = rel_k[ext_ids].T.astype(np.float16)  # [HD, 127]

    xq = f32("query")
    xk = f32("key")
    xv = f32("value")

    shared = {
        "w_q": _shuf_w(wq_t),
        "w_k": _shuf_w(wk_t),
        "w16r": np.stack([_shuf_w(wv_t), _shuf_w(wo_t)], axis=1),
        "w8x": np.stack(
            [_shuf_w(wgq8), _shuf_w(wgk8), _shuf_w(wgv8), _shuf_w(w2_8)], axis=1
        ),
        "relt": np.ascontiguousarray(relt),
    }
    in_maps = []
    for ci in range(NCORES):
        sl = slice(ci * BC, (ci + 1) * BC)
        x3 = np.stack(
            [
                xq[sl].reshape(NTOK, D).T,
                xk[sl].reshape(NTOK, D).T,
                xv[sl].reshape(NTOK, D).T,
            ]
        )
        in_maps.append(
            {"x16": _shuf_x(x3.astype(np.float16)), "x8": _shuf_x(f8(x3)), **shared}
        )
    return in_maps


def _run(inputs, **kwargs):
    nc = _get_module()
    in_maps = _prepare_in_maps(inputs)
    res = run_bass_kernel_spmd(nc, in_maps, core_ids=list(range(NCORES)), **kwargs)
    out = np.concatenate(
        [res.results[ci]["out"].reshape(BC, L, D) for ci in range(NCORES)], axis=0
    )
    out *= 1.0 / OUT_SCALE
    return out, res


def kernel(**inputs) -> np.ndarray:
    out, _ = _run(inputs)
    return out


def kernel_profiled(**inputs):
    out, res = _run(inputs, trace=True)
    return out, res



# revision 8
# speedup vs baseline: 1.2211x; 1.2211x over previous
"""Trainium2 Bass kernel for EnhancedMultiHeadAttention.

Data-parallel over batch: B=256 split as 32 batches per core across 8 cores.
Per core, 8 chunks of 256 tokens flow through a software pipeline.

Key optimizations over the fp16 baseline:
  - The global-branch projections (g_q, g_k, g_v) and the global half of the
    output projection run in fp8e4 (e4m3) with DoubleRow perf mode: two
    128-deep contraction tiles per pass, 2x PE throughput. Weights are
    pre-scaled x32/x64 host-side so e4m3 sees unit-scale data; descales are
    folded into the exp() scale and the v-projection drain.
  - All weights resident in SBUF, loaded once; every DMA-able tensor is
    pre-shuffled host-side into its exact SBUF layout so DMAs move long
    contiguous per-partition runs (strided descriptors are slow).
  - k-tiles and the rel-position table are stored adjacently per (head-tile,
    batch) so the qk matmul and the rel-bias matmul fuse into ONE 191-column
    matmul per stationary load (half the score-stage PE instructions).
  - Global AV runs one 128-wide-stationary matmul per (head, group) instead
    of two 64-wide halves.
  - Input x loads are prefetched one full chunk ahead on a dedicated DMA
    queue (mirrors/stores ride other queues) so the PE never waits on HBM.
  - Final projection is interleaved ts-block-wise into the softmax group
    loop so the tensor engine stays dense (it p-state-ramps: 2.4GHz only
    after ~3us of gapless execution).
  - attn^T comes from PE transposes arranged so every AV matmul runs with
    operands at PE-array row 0: odd-parity v batches are mirrored down to
    partitions 0:64 by one SBUF-SBUF DMA per chunk. (Back-to-back 64-row
    matmuls that alternate row position 0<->64 hang the hardware.)
  - Relative-position bias: T = q @ REL_EXT.T computed into the score PSUM,
    drained together with qk, stored once per chunk, and skew-added via a
    DRAM roundtrip with a diagonal (per-l shifted) read.

Output convention: PSUM of the final projection is x512 (wo pre-scaled);
the host wrapper divides by 512 after gathering.
"""

import sys

sys.path.insert(0, "/opt/trn_rl_repo")

import math
from contextlib import ExitStack

import numpy as np
import ml_dtypes

import concourse.bass as bass
import concourse.tile as tile
from concourse import bacc, mybir
from concourse.bass_utils import run_bass_kernel_spmd
from concourse.masks import make_identity

F16 = mybir.dt.float16
F32 = mybir.dt.float32
F8 = mybir.dt.float8e4
DR = mybir.MatmulPerfMode.DoubleRow
EXP = mybir.ActivationFunctionType.Exp
CPY = mybir.ActivationFunctionType.Copy

B, L, D = 256, 64, 1024
H, HD = 16, 64
HG, HDG = 8, 128
MAXREL = 32
NCORES = 8
BC = B // NCORES  # batches per core
NTOK = BC * L  # tokens per core
CH = 256  # tokens per chunk
NB = CH // L  # batches per chunk (4)
NCHUNK = NTOK // CH  # 8
KT = D // 128  # contraction tiles (8)
OT = D // 128  # output feature tiles (8)
NQ = CH * H // (2 * L)  # head-batch pairs per chunk (32)
NG = 2 * NB + NB  # softmax groups per chunk (8 local + 4 global)

SCALE_G = 1.0 / (1024.0 * math.sqrt(HDG))  # global exp descale
OUT_SCALE = 512.0  # final psum scale, host divides


def _emit(nc, tc, ctx, io):
    pconst = ctx.enter_context(tc.tile_pool(name="pconst", bufs=1))
    px = ctx.enter_context(tc.tile_pool(name="px", bufs=1))
    pact = ctx.enter_context(tc.tile_pool(name="pact", bufs=1))
    psmall = ctx.enter_context(tc.tile_pool(name="psmall", bufs=2))
    patt = ctx.enter_context(tc.tile_pool(name="patt", bufs=3))
    pout = ctx.enter_context(tc.tile_pool(name="pout", bufs=2))
    pp_proj = ctx.enter_context(tc.tile_pool(name="pp_proj", bufs=2, space="PSUM"))
    pp_big = ctx.enter_context(tc.tile_pool(name="pp_big", bufs=2, space="PSUM"))
    pp_sc = ctx.enter_context(tc.tile_pool(name="pp_sc", bufs=2, space="PSUM"))
    pp_av = ctx.enter_context(tc.tile_pool(name="pp_av", bufs=2, space="PSUM"))
    pdram = ctx.enter_context(tc.tile_pool(name="pdram", bufs=2, space="DRAM"))

    st = [dict() for _ in range(NCHUNK)]

    def load_x(c):
        # inputs ride the sync queue exclusively so prefetches never queue
        # behind mid-chunk SBUF traffic
        x16 = px.tile([128, 3, KT, CH], F16, tag="x16", name="x16", bufs=2)
        nc.sync.dma_start(x16[:], io["x16"][c])
        x8 = px.tile([128, 3, KT, CH], F8, tag="x8", name="x8", bufs=2)
        nc.sync.dma_start(x8[:], io["x8"][c])
        st[c]["x16"] = x16
        st[c]["x8"] = x8

    # ---- persistent weights on the scalar/gpsimd queues; x on sync ----
    load_x(0)
    w_q = pconst.tile([128, KT, D], F16, tag="w_q")
    nc.scalar.dma_start(w_q[:], io["w_q"][:])
    w_k = pconst.tile([128, KT, D], F16, tag="w_k")
    nc.scalar.dma_start(w_k[:], io["w_k"][:])
    w8x = pconst.tile([128, 4, KT, D], F8, tag="w8x")  # gq, gk, gv, w2
    nc.scalar.dma_start(w8x[:], io["w8x"][:])
    w16r = pconst.tile([128, 2, KT, D], F16, tag="w16r")  # v, o
    nc.gpsimd.dma_start(w16r[:], io["w16r"][:])
    relt = pconst.tile([128, 127], F16, tag="relt")
    nc.gpsimd.dma_start(relt[0:64, :], io["relt"][:])
    nc.gpsimd.dma_start(relt[64:128, :], io["relt"][:])
    ident = pconst.tile([128, 128], F16, tag="ident")
    make_identity(nc, ident[:])
    # k-tiles live next to a replicated rel table: one 191-col moving operand
    ktr = pconst.tile([128, OT, NB, 191], F16, tag="ktr")
    for j in range(OT):
        nc.vector.tensor_copy(
            out=ktr[:, j, :, 64:191],
            in_=relt[:, None, :].to_broadcast([128, NB, 127]),
        )
    load_x(1)
    wt = {
        "w_q": w_q,
        "w_k": w_k,
        "w_v": w16r[:, 0],
        "w_o": w16r[:, 1],
        "w_gq": w8x[:, 0],
        "w_gk": w8x[:, 1],
        "w_gv": w8x[:, 2],
        "w_2": w8x[:, 3],
    }

    # ---- projections ----
    def proj_t16_op(xt, w, op, dst, drain):
        # one column-pair of a transposed f16 projection
        ps = pp_proj.tile([128, 2, CH], F32, tag="pj")
        for ii in range(2):
            oi = 2 * op + ii
            for ki in range(KT):
                nc.tensor.matmul(
                    ps[:, ii, :],
                    w[:, ki, oi * 128 : (oi + 1) * 128],
                    xt[:, ki, :],
                    start=(ki == 0),
                    stop=(ki == KT - 1),
                )
        drain(ps)

    def proj_t8(xt8, w8, dst, drain):
        for op in range(OT // 2):
            ps = pp_proj.tile([128, 2, CH], F32, tag="pj")
            for ii in range(2):
                oi = 2 * op + ii
                for kp in range(KT // 2):
                    nc.tensor.matmul(
                        ps[:, ii, :],
                        w8[:, 2 * kp : 2 * kp + 2, oi * 128 : (oi + 1) * 128],
                        xt8[:, 2 * kp : 2 * kp + 2, :],
                        start=(kp == 0),
                        stop=(kp == KT // 2 - 1),
                        perf_mode=DR,
                    )
            drain(dst[:, 2 * op : 2 * op + 2, :], ps[:])

    def proj_n16(xt, w, dst):
        # dst[p, ts, o] f16: plain layout (tokens on partitions)
        for ts in range(NB // 2):
            for oh in range(2):
                ps = pp_big.tile([128, 512], F32, tag="mm")
                for ki in range(KT):
                    nc.tensor.matmul(
                        ps[:],
                        xt[:, ki, ts * 128 : (ts + 1) * 128],
                        w[:, ki, oh * 512 : (oh + 1) * 512],
                        start=(ki == 0),
                        stop=(ki == KT - 1),
                    )
                nc.scalar.copy(out=dst[:, ts, oh * 512 : (oh + 1) * 512], in_=ps[:])

    def proj_n8(xt8, w8, dst, scale):
        for ts in range(NB // 2):
            for oh in range(2):
                ps = pp_big.tile([128, 512], F32, tag="mm")
                for kp in range(KT // 2):
                    nc.tensor.matmul(
                        ps[:],
                        xt8[:, 2 * kp : 2 * kp + 2, ts * 128 : (ts + 1) * 128],
                        w8[:, 2 * kp : 2 * kp + 2, oh * 512 : (oh + 1) * 512],
                        start=(kp == 0),
                        stop=(kp == KT // 2 - 1),
                        perf_mode=DR,
                    )
                nc.scalar.activation(
                    dst[:, ts, oh * 512 : (oh + 1) * 512], ps[:], CPY, scale=scale
                )

    def drain_dve(dst, src):
        nc.vector.tensor_copy(out=dst, in_=src)

    def mirror(src, tag):
        # odd-parity batches' v rows moved down to partitions 0:64 so every
        # AV matmul can run at PE-array row 0
        dup = pact.tile([64, NB // 2, D], F16, tag=tag, name=tag)
        nc.scalar.dma_start(dup[:], src[64:128, :, :])
        return dup

    # ---- q/k projections interleaved with local scores ----
    def scores_group(c, p):
        # scores for head-tiles j = 2p, 2p+1 over all batches; qk and rel
        # bias in one 191-col matmul per stationary q load
        qt, qksb = st[c]["qt"], st[c]["qksb"]
        for b in range(NB):
            bcols = slice(b * L, (b + 1) * L)
            sc = pp_sc.tile([128, 2, 256], F32, tag="sc")
            for u in range(2):
                j = 2 * p + u
                nc.tensor.matmul(
                    sc[0:64, u, 0:191],
                    qt[0:64, j, bcols],
                    ktr[0:64, j, b, :],
                    start=True,
                    stop=True,
                )
                nc.tensor.matmul(
                    sc[64:128, u, 0:191],
                    qt[64:128, j, bcols],
                    ktr[64:128, j, b, :],
                    start=True,
                    stop=True,
                )
            col = b * 8 + 2 * p
            if b % 2:
                nc.scalar.copy(out=qksb[:, col : col + 2, :], in_=sc[:, :, 0:191])
            else:
                nc.vector.tensor_copy(out=qksb[:, col : col + 2, :], in_=sc[:, :, 0:191])

    def proj_qk_scores(c):
        x16 = st[c]["x16"]
        qt = pact.tile([128, OT, CH], F16, tag="qt", name="qt")
        qksb = pact.tile([128, NQ, 191], F16, tag="qksb", name="qksb")
        st[c].update(qt=qt, qksb=qksb)
        for p in range(OT // 2):
            proj_t16_op(
                x16[:, 0], wt["w_q"], p, qt,
                lambda ps, _p=p: drain_dve(qt[:, 2 * _p : 2 * _p + 2, :], ps[:]),
            )
            proj_t16_op(
                x16[:, 1], wt["w_k"], p, None,
                lambda ps, _p=p: drain_dve(
                    ktr[:, 2 * _p : 2 * _p + 2, :, 0:64],
                    ps[:].rearrange("p i (b t) -> p i b t", b=NB),
                ),
            )
            if p:
                scores_group(c, p - 1)
        scores_group(c, OT // 2 - 1)

    def proj_rest(c):
        x16, x8 = st[c]["x16"], st[c]["x8"]
        qgt = pact.tile([128, OT, CH], F16, tag="qgt", name="qgt")
        proj_t8(x8[:, 0], wt["w_gq"], qgt, drain_dve)
        kgt = pact.tile([128, OT, CH], F16, tag="kgt", name="kgt")
        proj_t8(x8[:, 1], wt["w_gk"], kgt, drain_dve)
        vt = pact.tile([128, NB // 2, D], F16, tag="vt", name="vt")
        proj_n16(x16[:, 2], wt["w_v"], vt)
        vgt = pact.tile([128, NB // 2, D], F16, tag="vgt", name="vgt")
        proj_n8(x8[:, 2], wt["w_gv"], vgt, 0.25)
        st[c].update(qgt=qgt, kgt=kgt, vt=vt, vgt=vgt,
                     vdup=mirror(vt, "vdup"), vgdup=mirror(vgt, "vgdup"))

    def scores_store(c):
        qksb = st[c]["qksb"]
        tdr = pdram.tile([2, 64, NQ, 127], F16, tag="tdr")
        st[c]["tdr"] = tdr
        tap = tdr[:]
        # one T store for the whole chunk:
        # src qksb[(pair,l), q, 64:191] -> tdr[pair, l, q, j] ((q,j) contiguous)
        dst = bass.AP(
            tap.tensor,
            tap.offset,
            [[64 * NQ * 127, 2], [NQ * 127, 64], [1, NQ * 127]],
        )
        nc.scalar.dma_start(dst, qksb[:, :, 64:191])

    def skew_read(c):
        qksb, tdr = st[c]["qksb"], st[c]["tdr"]
        tap = tdr[:]
        # skew-read: qksb[pair*64+l, q, r] += tdr[pair, l, q, r-l+63]
        for pair in range(2):
            src = bass.AP(
                tap.tensor,
                tap.offset + 63 + pair * 64 * NQ * 127,
                [[NQ * 127 - 1, 64], [127, NQ], [1, 64]],
            )
            nc.gpsimd.dma_start(
                qksb[pair * 64 : (pair + 1) * 64, :, 0:64],
                src,
                accum_op=mybir.AluOpType.add,
            )

    def scores_global(c):
        qgt, kgt = st[c]["qgt"], st[c]["kgt"]
        gsb = pact.tile([128, 4 * NB, 64], F16, tag="gsb", name="gsb")
        for g in range(NB):
            bcols = slice(g * L, (g + 1) * L)
            sc = pp_av.tile([128, 512], F32, tag="av")
            for i in range(4):
                for pair in range(2):
                    hg = 2 * i + pair
                    nc.tensor.matmul(
                        sc[pair * 64 : (pair + 1) * 64, i * 64 : (i + 1) * 64],
                        qgt[:, hg, bcols],
                        kgt[:, hg, bcols],
                        start=True,
                        stop=True,
                    )
            nc.scalar.copy(
                out=gsb[:, 4 * g : 4 * g + 4, :],
                in_=sc[:, 0:256].rearrange("p (i c) -> p i c", i=4),
            )
        st[c]["gsb"] = gsb

    # ---- softmax + AV, one group (4 q-pairs) at a time ----
    def softmax_block(src_ap, scale, av):
        # av: this group's [128, 512] f32 psum bank; cols 256:512 (viewed as
        # f16 cols 512:1024) hold the 4 transposed-attn tiles [64, 128].
        esb = psmall.tile([128, 4, 64], F16, tag="esb")
        if scale is None:
            nc.scalar.activation(esb[:], src_ap, EXP)
        else:
            nc.scalar.activation(esb[:], src_ap, EXP, scale=scale)
        sums = psmall.tile([128, 4], F32, tag="sums")
        nc.vector.tensor_reduce(sums[:], esb[:], mybir.AxisListType.X,
                                mybir.AluOpType.add)
        rcp = psmall.tile([128, 4], F32, tag="rcp")
        nc.vector.reciprocal(rcp[:], sums[:])
        attn = psmall.tile([128, 4, 64], F16, tag="attn")
        nc.gpsimd.tensor_tensor(
            attn[:], esb[:], rcp[:, :, None].to_broadcast([128, 4, 64]),
            mybir.AluOpType.mult,
        )
        trv = av[:].bitcast(F16)  # [128, 1024] f16 view of the bank
        for i in range(4):
            nc.tensor.transpose(
                trv[0:64, 512 + i * 128 : 512 + (i + 1) * 128],
                attn[:, i, :],
                ident[:],
            )
        # att4[r, i, l2]: i-th q-pair's attn^T, [64 r, 128 (lo-l | hi-l)]
        att4 = patt.tile([64, 4, 128], F16, tag="att4", name="att4")
        nc.scalar.copy(
            out=att4[:],
            in_=trv[0:64, 512:1024].rearrange("p (i c) -> p i c", i=4),
        )
        return att4

    def smx_local_g(c, g):
        qksb, vt, vdup, lt = st[c]["qksb"], st[c]["vt"], st[c]["vdup"], st[c]["lt"]
        b = g // 2
        j0 = 4 * (g % 2)
        bcols = slice(b * L, (b + 1) * L)
        av = pp_av.tile([128, 512], F32, tag="av")
        att4 = softmax_block(qksb[:, 4 * g : 4 * g + 4, 0:64], None, av)
        vsrc = vt if (b % 2) == 0 else vdup
        for i in range(4):
            for pair in range(2):
                h = 2 * (j0 + i) + pair
                nc.tensor.matmul(
                    av[pair * 64 : (pair + 1) * 64, i * 64 : (i + 1) * 64],
                    vsrc[0:64, b // 2, h * 64 : (h + 1) * 64],
                    att4[:, i, pair * 64 : (pair + 1) * 64],
                    start=True,
                    stop=True,
                )
        nc.vector.tensor_copy(
            out=lt[:, j0 : j0 + 4, bcols],
            in_=av[:, 0:256].rearrange("p (i c) -> p i c", i=4),
        )

    def smx_global_g(c, g):
        gsb, vgt, vgdup, gt = st[c]["gsb"], st[c]["vgt"], st[c]["vgdup"], st[c]["gt"]
        b = g
        bcols = slice(b * L, (b + 1) * L)
        av0 = pp_av.tile([128, 512], F32, tag="av")
        att4 = softmax_block(gsb[:, 4 * g : 4 * g + 4, :], SCALE_G, av0)
        vsrc = vgt if (b % 2) == 0 else vgdup
        for half in range(2):
            av = av0 if half == 0 else pp_av.tile([128, 512], F32, tag="av")
            for k in range(4):
                hg = 4 * half + k
                i, pair = hg // 2, hg % 2
                nc.tensor.matmul(
                    av[:, k * 64 : (k + 1) * 64],
                    vsrc[0:64, b // 2, hg * 128 : (hg + 1) * 128],
                    att4[:, i, pair * 64 : (pair + 1) * 64],
                    start=True,
                    stop=True,
                )
            nc.vector.tensor_copy(
                out=gt[:, 4 * half : 4 * half + 4, bcols],
                in_=av[:, 0:256].rearrange("p (k c) -> p k c", k=4),
            )

    def smx_alloc(c):
        st[c]["lt"] = pact.tile([128, KT, CH], F16, tag="lt", name="lt")
        st[c]["gt"] = pact.tile([128, KT, CH], F8, tag="gt", name="gt")

    def smx_g(c, m):
        if m < 2 * NB:
            smx_local_g(c, m)
        else:
            smx_global_g(c, m - 2 * NB)

    def final_ts(c, ts):
        tok0 = c * CH
        lt, gt = st[c]["lt"], st[c]["gt"]
        osb = pout.tile([128, 1024], F32, tag="out")
        for oh in range(2):
            ps = pp_big.tile([128, 512], F32, tag="mm")
            for ki in range(KT):
                nc.tensor.matmul(
                    ps[:],
                    lt[:, ki, ts * 128 : (ts + 1) * 128],
                    wt["w_o"][:, ki, oh * 512 : (oh + 1) * 512],
                    start=(ki == 0),
                    stop=False,
                )
            for kp in range(KT // 2):
                nc.tensor.matmul(
                    ps[:],
                    gt[:, 2 * kp : 2 * kp + 2, ts * 128 : (ts + 1) * 128],
                    wt["w_2"][:, 2 * kp : 2 * kp + 2, oh * 512 : (oh + 1) * 512],
                    start=False,
                    stop=(kp == KT // 2 - 1),
                    perf_mode=DR,
                )
            nc.vector.tensor_copy(out=osb[:, oh * 512 : (oh + 1) * 512], in_=ps[:])
        nc.scalar.dma_start(
            io["out"][tok0 + ts * 128 : tok0 + (ts + 1) * 128, :], osb[:]
        )
        if ts == NB // 2 - 1:
            st[c].clear()

    def smx_final(c):
        # softmax groups interleaved with the final projection so PE work
        # fills the scalar/vector softmax chain latency
        for m in (0, 1, 2, 3, 8, 9):
            smx_g(c, m)
        final_ts(c, 0)
        for m in (4, 5, 6, 7, 10, 11):
            smx_g(c, m)
        final_ts(c, 1)

    # ---- pipeline ----
    proj_qk_scores(0)
    scores_store(0)
    skew_read(0)
    proj_rest(0)
    scores_global(0)
    for c in range(1, NCHUNK):
        if c + 1 < NCHUNK:
            load_x(c + 1)
        proj_qk_scores(c)
        smx_alloc(c - 1)
        scores_store(c)
        smx_final(c - 1)
        skew_read(c)
        proj_rest(c)
        scores_global(c)
    smx_alloc(NCHUNK - 1)
    smx_final(NCHUNK - 1)


_NC_CACHE = {}


def _get_module():
    if "nc" not in _NC_CACHE:
        nc = bacc.Bacc("TRN2", target_bir_lowering=False, debug=False)
        io = {}
        io["x16"] = nc.dram_tensor(
            "x16", [NCHUNK, 128, 3, KT, CH], F16, kind="ExternalInput"
        ).ap()
        io["x8"] = nc.dram_tensor(
            "x8", [NCHUNK, 128, 3, KT, CH], F8, kind="ExternalInput"
        ).ap()
        io["w_q"] = nc.dram_tensor("w_q", [128, KT, D], F16, kind="ExternalInput").ap()
        io["w_k"] = nc.dram_tensor("w_k", [128, KT, D], F16, kind="ExternalInput").ap()
        io["w16r"] = nc.dram_tensor(
            "w16r", [128, 2, KT, D], F16, kind="ExternalInput"
        ).ap()
        io["w8x"] = nc.dram_tensor(
            "w8x", [128, 4, KT, D], F8, kind="ExternalInput"
        ).ap()
        io["relt"] = nc.dram_tensor("relt", [64, 127], F16, kind="ExternalInput").ap()
        io["out"] = nc.dram_tensor("out", [NTOK, D], F32, kind="ExternalOutput").ap()
        with tile.TileContext(nc) as tc, ExitStack() as ctx:
            _emit(nc, tc, ctx, io)
        nc.compile()
        _NC_CACHE["nc"] = nc
    return _NC_CACHE["nc"]


def _shuf_w(w):
    # [D(=ki*128+p), O] -> [p, ki, O]
    return np.ascontiguousarray(w.reshape(KT, 128, -1).transpose(1, 0, 2))


def _shuf_x(x3):
    # [3, D(=ki*128+p), NTOK(=c*CH+t)] -> [c, p, 3, ki, t]
    return np.ascontiguousarray(
        x3.reshape(3, KT, 128, NCHUNK, CH).transpose(3, 2, 0, 1, 4)
    )


def _prepare_in_maps(inputs):
    f32 = lambda name: np.asarray(inputs[name], np.float32)
    f8 = lambda a: a.astype(ml_dtypes.float8_e4m3fn)

    S = 32.0
    wq_t = f32("Wq").T.astype(np.float16)
    wk_t = (f32("Wk").T / math.sqrt(HD)).astype(np.float16)
    wv_t = f32("Wv").T.astype(np.float16)
    g_in = f32("g_in_w")
    wgq8 = f8(g_in[0:D].T * S)
    wgk8 = f8(g_in[D : 2 * D].T * S)  # no sqrt fold; folded into SCALE_G
    wgv8 = f8(g_in[2 * D : 3 * D].T * S)
    wo = f32("Wo")
    gow = f32("g_out_w")
    wo_t = (0.7 * OUT_SCALE * wo.T).astype(np.float16)
    w2_8 = f8(0.3 * 64.0 * (gow.T @ wo.T))  # gt is x8 => 8*64 = OUT_SCALE

    for bname in ("bq", "bk", "bv", "bo", "g_in_b", "g_out_b"):
        assert not np.any(f32(bname)), f"nonzero bias {bname} not supported"

    rel_k = f32("rel_k")
    ext_ids = np.clip(np.arange(127) - 63, -MAXREL, MAXREL) + MAXREL
    relt = rel_k[ext_ids].T.astype(np.float16)  # [HD, 127]

    xq = f32("query")
    xk = f32("key")
    xv = f32("value")

    shared = {
        "w_q": _shuf_w(wq_t),
        "w_k": _shuf_w(wk_t),
        "w16r": np.stack([_shuf_w(wv_t), _shuf_w(wo_t)], axis=1),
        "w8x": np.stack(
            [_shuf_w(wgq8), _shuf_w(wgk8), _shuf_w(wgv8), _shuf_w(w2_8)], axis=1
        ),
        "relt": np.ascontiguousarray(relt),
    }
    in_maps = []
    for ci in range(NCORES):
        sl = slice(ci * BC, (ci + 1) * BC)
        x3 = np.stack(
            [
                xq[sl].reshape(NTOK, D).T,
                xk[sl].reshape(NTOK, D).T,
                xv[sl].reshape(NTOK, D).T,
            ]
        )
        in_maps.append(
            {"x16": _shuf_x(x3.astype(np.float16)), "x8": _shuf_x(f8(x3)), **shared}
        )
    return in_maps


def _run(inputs, **kwargs):
    nc = _get_module()
    in_maps = _prepare_in_maps(inputs)
    res = run_bass_kernel_spmd(nc, in_maps, core_ids=list(range(NCORES)), **kwargs)
    out = np.concatenate(
        [res.results[ci]["out"].reshape(BC, L, D) for ci in range(NCORES)], axis=0
    )
    out *= 1.0 / OUT_SCALE
    return out, res


def kernel(**inputs) -> np.ndarray:
    out, _ = _run(inputs)
    return out


def kernel_profiled(**inputs):
    out, res = _run(inputs, trace=True)
    return out, res
